# revision 12
# baseline (speedup 1.0000x reference)
"""Trainium2 Bass kernel for nn_BiasVectorsBlock (MVN sampling block).

Computes, for x [32, 2048, 512] and z [32, 512]:
    mean = mean(x, axis=(0,1))
    cov  = mean_b( xc_b^T xc_b / (T-1) ),  xc_b = x_b - mean_t(x_b)
    L    = cholesky(cov);  out = mean + z @ L^T

Strategy (8 NeuronCores, data-parallel over B):
  - core c DMA-loads its 4 batches once, casting f32 -> bf16 in the DMA.
    TensorE accumulates the Gram matrix G_c = sum x^T x (upper-triangle
    strips) and per-batch column sums S_c (selector-column matmuls) in
    PSUM across all 64 chunks.
  - correction -S_c^T S_c / T is accumulated into the same PSUM banks, so
    PSUM holds sum_b (x_b - mean_b)^T (x_b - mean_b) for the core's
    batches (bf16 inputs, fp32 accumulation).
  - subtract (T-1)*B/8 * I so the AllReduce payload is zero-centered
    (bf16-safe), pack to bf16, one AllReduce (~330 KB).
  - every core computes E = cov - I and runs the sqrt-free Cholesky
    fixed-point iteration  Y <- Phi_u(E - Y^T Y)  (Phi_u = strict upper +
    half diagonal; at the fixed point cov = (I+Y)^T (I+Y), i.e. R = I+Y
    is the upper Cholesky factor; converges in ~5 rounds since
    ||E||_2 ~ 0.18).  Rounds 0-3 in bf16 with E folded into PSUM via an
    identity matmul; the last round re-solves against fp32 E on VectorE.
  - out = z + (z @ Y) + mean via 4 fp32 z^T-chunk matmuls + a K=1
    ones-matmul that broadcasts mean into PSUM.
"""

import os
import sys

for _p in ("/opt/trn_rl_repo",):
    if _p not in sys.path and os.path.isdir(_p):
        sys.path.insert(0, _p)

import numpy as np

B, T, D = 32, 2048, 512
NCORES = 8
BC = B // NCORES          # batches per core
CH = T // 128             # 128-row chunks per batch
DENOM = (T - 1) * B       # cov denominator
SHIFT = DENOM / NCORES    # identity shift per core, so AR payload is zero-mean
W = [512, 384, 256, 128]  # upper-strip widths (strip i: rows 128i.., cols 128i..512)
N_BF16_ROUNDS = 4
AR_COLS = sum(W)          # 1280 packed columns


def _build_nc():
    import concourse.bacc as bacc
    import concourse.mybir as mybir
    import ml_dtypes
    from concourse.tile import TileContext

    f32 = mybir.dt.float32
    bf16 = mybir.dt.bfloat16
    mult = mybir.AluOpType.mult

    # Bacc (not raw Bass): its generate_event_semaphores pass splits
    # multi-wait instructions, which DMA opcodes require on TRN2.
    nc = bacc.Bacc(None, num_devices=NCORES)

    x_in = nc.declare_dram_parameter("x", [BC, T, D], f32, isOutput=False)
    z_in = nc.declare_dram_parameter("z", [B, D], f32, isOutput=False)
    zt_in = nc.declare_dram_parameter("zt", [D, B], f32, isOutput=False)
    out_ext = nc.declare_dram_parameter("out", [B, D], f32, isOutput=True)

    # ---- constants (embedded in the NEFF) ----
    # -Phi mask, shared by all strips: local cols 0:128 hold the diagonal
    # block (strict-upper -> -1, diag -> -0.5, lower -> 0); cols 128:512 -> -1.
    m = np.zeros((128, 512), np.float32)
    m[:, 128:] = -1.0
    r, c = np.indices((128, 128))
    m[:, :128] = np.where(c > r, -1.0, np.where(c == r, -0.5, 0.0)).astype(np.float32)
    maskneg_d = nc.inline_tensor(m, name="maskneg")

    eye = np.eye(128, dtype=np.float32)
    eyeb_d = nc.inline_tensor(eye.astype(ml_dtypes.bfloat16), name="eyeb")
    negshifti_d = nc.inline_tensor((-SHIFT) * eye, name="negshifti")
    sel = np.zeros((128, 4 * BC), np.float32)
    for b in range(BC):
        sel[:, 4 * b + b] = 1.0  # batch b's ones-column -> psum row b
    sel4_d = nc.inline_tensor(sel.astype(ml_dtypes.bfloat16), name="sel4")
    ones4_d = nc.inline_tensor(np.ones((BC, 1), ml_dtypes.bfloat16), name="ones4")
    ones1x32_d = nc.inline_tensor(np.ones((1, B), np.float32), name="ones1x32")

    rg = [list(range(NCORES))]

    with TileContext(nc) as tc, \
            tc.tile_pool(name="sb", bufs=1) as sb, \
            tc.tile_pool(name="dr", space="DRAM", bufs=1) as dr:

        # consts to SBUF
        maskneg = sb.tile_from(maskneg_d[:, :], name="maskneg_sb")
        eyeb = sb.tile_from(eyeb_d[:, :], name="eyeb_sb")
        negshifti = sb.tile_from(negshifti_d[:, :], name="negshifti_sb")
        sel4 = sb.tile_from(sel4_d[:, :], name="sel4_sb")
        ones4 = sb.tile_from(ones4_d[:, :], name="ones4_sb")
        ones1x32 = sb.tile_from(ones1x32_d[:, :], name="ones1x32_sb")

        z_sb = sb.tile([B, D], f32, name="z_sb")
        nc.sync.dma_start(out=z_sb[:, :], in_=z_in[:, :])
        zts = []
        for k in range(4):
            zt_k = sb.tile([128, B], f32, name=f"zt{k}_sb")
            nc.sync.dma_start(out=zt_k[:, :], in_=zt_in[k * 128:(k + 1) * 128, :])
            zts.append(zt_k)

        # ---- phase A: Gram strips + per-batch column sums ----
        with tc.tile_pool(name="psA", space="PSUM", bufs=1) as ps:
            g = [ps.tile([128, W[i]], f32, tag=f"g{i}", bufs=1, name=f"g{i}")
                 for i in range(4)]
            srow = ps.tile([BC, D], f32, tag="srow", bufs=1, name="srow")
            for b in range(BC):
                # sync (HWDGE) DMA in f32, then GpSimd converts to bf16
                # (1-input GpSimd ops run at line rate; DVE stays free).
                xf = sb.tile([128, CH * D], f32, tag="xf", bufs=2, name=f"xf{b}")
                xb = sb.tile([128, CH * D], bf16, tag="xb", bufs=2, name=f"xb{b}")
                xf3 = xf.rearrange("p (c d) -> p c d", d=D)
                xs3 = x_in[b].rearrange("(c p) d -> p c d", p=128)
                half = CH // 2
                nc.sync.dma_start(out=xf3[:, :half, :], in_=xs3[:, :half, :])
                nc.sync.dma_start(out=xf3[:, half:, :], in_=xs3[:, half:, :])
                q = CH * D // 4
                for qi in range(4):
                    nc.gpsimd.tensor_copy(out=xb[:, qi * q:(qi + 1) * q],
                                          in_=xf[:, qi * q:(qi + 1) * q])
                for cch in range(CH):
                    xc = xb[:, cch * D:(cch + 1) * D]
                    for i in range(4):
                        nc.tensor.matmul(
                            g[i][:, :],
                            lhsT=xc[:, i * 128:(i + 1) * 128],
                            rhs=xc[:, 128 * i:],
                            start=(b == 0 and cch == 0), stop=False,
                        )
                    nc.tensor.matmul(
                        srow[:, :],
                        lhsT=sel4[:, 4 * b:4 * (b + 1)],
                        rhs=xc,
                        start=(b == 0 and cch == 0),
                        stop=(b == BC - 1 and cch == CH - 1),
                    )

            s_bf = sb.tile([BC, D], bf16, name="s_bf")
            nc.vector.tensor_copy(out=s_bf[:, :], in_=srow[:, :])
            sneg = sb.tile([BC, D], bf16, name="sneg")
            nc.vector.tensor_scalar_mul(sneg[:, :], srow[:, :], -1.0 / T)
            for i in range(4):
                nc.tensor.matmul(
                    g[i][:, :],
                    lhsT=sneg[:, i * 128:(i + 1) * 128],
                    rhs=s_bf[:, 128 * i:],
                    start=False, stop=True,
                )
            mrow = ps.tile([1, D], f32, tag="mrow", bufs=1, name="mrow")
            nc.tensor.matmul(mrow[:, :], lhsT=ones4[:, :], rhs=s_bf[:, :],
                             start=True, stop=True)

            # pack (PSUM - shift*I) to bf16
            arin_sb = sb.tile([128, AR_COLS], bf16, name="arin_sb")
            for i in range(4):
                cs = sum(W[:i])
                nc.vector.tensor_add(
                    out=arin_sb[:, cs:cs + 128],
                    in0=g[i][:, 0:128],
                    in1=negshifti[:, :],
                )
                if W[i] > 128:
                    nc.vector.tensor_copy(
                        out=arin_sb[:, cs + 128:cs + W[i]],
                        in_=g[i][:, 128:W[i]],
                    )
            arm_sb = sb.tile([1, AR_COLS], bf16, name="arm_sb")
            nc.vector.memset(arm_sb[:, D:], 0.0)
            nc.vector.tensor_copy(out=arm_sb[:, 0:D], in_=mrow[:, :])

        # ---- AllReduce ----
        ar_in = dr.tile([129, AR_COLS], bf16, name="ar_in")
        ar_out = dr.tile([129, AR_COLS], bf16, addr_space="Shared", name="ar_out")
        nc.sync.dma_start(out=ar_in[0:128, :], in_=arin_sb[:, :])
        nc.sync.dma_start(out=ar_in[128:129, :], in_=arm_sb[:, :])
        nc.gpsimd.collective_compute(
            "AllReduce",
            mybir.AluOpType.add,
            replica_groups=rg,
            ins=[ar_in[:, :].opt()],
            outs=[ar_out[:, :].opt()],
        )

        # ---- unpack: -E strips in bf16, masked fp32 E for the final round ----
        ebn, em = [], []
        for i in range(4):
            cs = sum(W[:i])
            er = sb.tile([128, W[i]], bf16, name=f"er{i}")
            nc.sync.dma_start(out=er[:, :], in_=ar_out[0:128, cs:cs + W[i]])
            eb = sb.tile([128, W[i]], bf16, name=f"ebn{i}")
            nc.vector.tensor_scalar_mul(eb[:, :], er[:, :], -1.0 / DENOM)
            # em_i = (E/DENOM's scale) * (-mask) = -E*mask  (one fused DVE op)
            ef = sb.tile([128, W[i]], f32, name=f"em{i}")
            nc.vector.scalar_tensor_tensor(
                out=ef[:, :], in0=er[:, :], scalar=1.0 / DENOM,
                in1=maskneg[:, :W[i]], op0=mult, op1=mult)
            ebn.append(eb)
            em.append(ef)
        armo = sb.tile([1, D], bf16, name="armo")
        nc.sync.dma_start(out=armo[:, :], in_=ar_out[128:129, 0:D])
        mean_sb = sb.tile([1, D], f32, name="mean_sb")
        nc.vector.tensor_scalar_mul(mean_sb[:, :], armo[:, :], 1.0 / (B * T))

        # ---- phase B: Cholesky fixed-point iteration + affine ----
        with tc.tile_pool(name="psB", space="PSUM", bufs=1) as ps:
            Y = None
            for rnd in range(N_BF16_ROUNDS + 1):
                last = rnd == N_BF16_ROUNDS
                newY = []
                for i in range(4):
                    p = ps.tile([128, W[i]], f32, tag="it", bufs=4,
                                name=f"it{rnd}_{i}")
                    first = True
                    if Y is not None:
                        for k in range(i + 1):
                            lo = 128 * (i - k)
                            nc.tensor.matmul(
                                p[:, :],
                                lhsT=Y[k][:, lo:lo + 128],
                                rhs=Y[k][:, lo:],
                                start=first, stop=(last and k == i),
                            )
                            first = False
                    if not last:
                        # fold -E into the accumulation via identity matmul
                        nc.tensor.matmul(p[:, :], lhsT=eyeb[:, :],
                                         rhs=ebn[i][:, :],
                                         start=first, stop=True)
                        ny = sb.tile([128, W[i]], bf16, tag="y", bufs=8,
                                     name=f"y{rnd}_{i}")
                        # psum = Y^T Y - E;  Y_new = -Phi(psum) = psum * (-mask)
                        nc.vector.tensor_tensor(out=ny[:, :], in0=p[:, :],
                                                in1=maskneg[:, :W[i]], op=mult)
                    else:
                        # fp32 refinement: Y = (E - Y^T Y) * mask
                        #   = psum*(-mask) - (-E*mask) = psum*(-mask) - em
                        ny = sb.tile([128, W[i]], f32, tag="yf", bufs=4,
                                     name=f"y{rnd}_{i}")
                        nc.vector.tensor_tensor(out=ny[:, :], in0=p[:, :],
                                                in1=maskneg[:, :W[i]], op=mult)
                        nc.vector.tensor_sub(ny[:, :], ny[:, :], em[i][:, :])
                    newY.append(ny)
                Y = newY

            # affine: out = z + z @ Y + mean  (fp32 matmuls; cheap)
            aff = ps.tile([B, D], f32, tag="aff", bufs=1, name="aff")
            for k in range(4):
                nc.tensor.matmul(
                    aff[:, 128 * k:],
                    lhsT=zts[k][:, :],
                    rhs=Y[k][:, :],
                    start=(k == 0), stop=False,
                )
            nc.tensor.matmul(aff[:, :], lhsT=ones1x32[:, :], rhs=mean_sb[:, :],
                             start=False, stop=True)
            out_sb = sb.tile([B, D], f32, name="out_sb")
            nc.vector.tensor_add(out=out_sb[:, :], in0=aff[:, :], in1=z_sb[:, :])
            nc.sync.dma_start(out=out_ext[:, :], in_=out_sb[:, :])

    nc.finalize()  # Bacc: runs event-sem splitting + register allocation
    return nc


_NC_CACHE = {}


def _get_nc():
    if "nc" not in _NC_CACHE:
        _NC_CACHE["nc"] = _build_nc()
    return _NC_CACHE["nc"]


def _in_maps(x, z):
    zt = np.ascontiguousarray(z.T)
    return [
        {"x": np.ascontiguousarray(x[c * BC:(c + 1) * BC]), "z": z, "zt": zt}
        for c in range(NCORES)
    ]


def kernel(x: np.ndarray, z: np.ndarray) -> np.ndarray:
    from concourse.bass_utils import run_bass_kernel_spmd

    x = np.ascontiguousarray(np.asarray(x, dtype=np.float32))
    z = np.ascontiguousarray(np.asarray(z, dtype=np.float32))
    nc = _get_nc()
    res = run_bass_kernel_spmd(nc, _in_maps(x, z), core_ids=list(range(NCORES)))
    return np.asarray(res.results[0]["out"], dtype=np.float32)


# revision 13
# speedup vs baseline: 1.3320x; 1.3320x over previous
"""Trainium2 Bass kernel for nn_BiasVectorsBlock (MVN sampling block).

Computes, for x [32, 2048, 512] and z [32, 512]:
    mean = mean(x, axis=(0,1))
    cov  = mean_b( xc_b^T xc_b / (T-1) ),  xc_b = x_b - mean_t(x_b)
    L    = cholesky(cov);  out = mean + z @ L^T

Strategy (8 NeuronCores, data-parallel over B):
  - core c DMA-loads its 4 batches once, casting f32 -> bf16 in the DMA.
    TensorE accumulates the Gram matrix G_c = sum x^T x (upper-triangle
    strips) and per-batch column sums S_c (selector-column matmuls) in
    PSUM across all 64 chunks.
  - correction -S_c^T S_c / T is accumulated into the same PSUM banks, so
    PSUM holds sum_b (x_b - mean_b)^T (x_b - mean_b) for the core's
    batches (bf16 inputs, fp32 accumulation).
  - subtract (T-1)*B/8 * I so the AllReduce payload is zero-centered
    (bf16-safe), pack to bf16, one AllReduce (~330 KB).
  - every core computes E = cov - I and runs the sqrt-free Cholesky
    fixed-point iteration  Y <- Phi_u(E - Y^T Y)  (Phi_u = strict upper +
    half diagonal; at the fixed point cov = (I+Y)^T (I+Y), i.e. R = I+Y
    is the upper Cholesky factor; converges in ~5 rounds since
    ||E||_2 ~ 0.18).  Rounds 0-3 in bf16 with E folded into PSUM via an
    identity matmul; the last round re-solves against fp32 E on VectorE.
  - out = z + (z @ Y) + mean via 4 fp32 z^T-chunk matmuls + a K=1
    ones-matmul that broadcasts mean into PSUM.
"""

import os
import sys

for _p in ("/opt/trn_rl_repo",):
    if _p not in sys.path and os.path.isdir(_p):
        sys.path.insert(0, _p)

import numpy as np

B, T, D = 32, 2048, 512
NCORES = 8
BC = B // NCORES          # batches per core
CH = T // 128             # 128-row chunks per batch
DENOM = (T - 1) * B       # cov denominator
SHIFT = DENOM / NCORES    # identity shift per core, so AR payload is zero-mean
W = [512, 384, 256, 128]  # upper-strip widths (strip i: rows 128i.., cols 128i..512)
N_BF16_ROUNDS = 4
AR_COLS = sum(W)          # 1280 packed columns


def _build_nc():
    import concourse.bacc as bacc
    import concourse.mybir as mybir
    import ml_dtypes
    from concourse.tile import TileContext

    f32 = mybir.dt.float32
    bf16 = mybir.dt.bfloat16
    mult = mybir.AluOpType.mult

    # Bacc (not raw Bass): its generate_event_semaphores pass splits
    # multi-wait instructions, which DMA opcodes require on TRN2.
    nc = bacc.Bacc(None, num_devices=NCORES)

    x_in = nc.declare_dram_parameter("x", [BC, T, D], f32, isOutput=False)
    z_in = nc.declare_dram_parameter("z", [B, D], f32, isOutput=False)
    zt_in = nc.declare_dram_parameter("zt", [D, B], f32, isOutput=False)
    out_ext = nc.declare_dram_parameter("out", [B, D], f32, isOutput=True)

    # ---- constants (embedded in the NEFF) ----
    # -Phi mask, shared by all strips: local cols 0:128 hold the diagonal
    # block (strict-upper -> -1, diag -> -0.5, lower -> 0); cols 128:512 -> -1.
    m = np.zeros((128, 512), np.float32)
    m[:, 128:] = -1.0
    r, c = np.indices((128, 128))
    m[:, :128] = np.where(c > r, -1.0, np.where(c == r, -0.5, 0.0)).astype(np.float32)
    maskneg_d = nc.inline_tensor(m, name="maskneg")

    eye = np.eye(128, dtype=np.float32)
    eyeb_d = nc.inline_tensor(eye.astype(ml_dtypes.bfloat16), name="eyeb")
    negshifti_d = nc.inline_tensor((-SHIFT) * eye, name="negshifti")
    sel = np.zeros((128, 4 * BC), np.float32)
    for b in range(BC):
        sel[:, 4 * b + b] = 1.0  # batch b's ones-column -> psum row b
    sel4_d = nc.inline_tensor(sel.astype(ml_dtypes.bfloat16), name="sel4")
    ones4_d = nc.inline_tensor(np.ones((BC, 1), ml_dtypes.bfloat16), name="ones4")
    ones1x32_d = nc.inline_tensor(np.ones((1, B), np.float32), name="ones1x32")

    rg = [list(range(NCORES))]

    with TileContext(nc) as tc, \
            tc.tile_pool(name="sb", bufs=1) as sb, \
            tc.tile_pool(name="dr", space="DRAM", bufs=1) as dr:

        # ---- phase A: Gram strips + per-batch column sums ----
        with tc.tile_pool(name="psA", space="PSUM", bufs=1) as ps:
            g = [ps.tile([128, W[i]], f32, tag=f"g{i}", bufs=1, name=f"g{i}")
                 for i in range(4)]
            srow = ps.tile([BC, D], f32, tag="srow", bufs=1, name="srow")
            for b in range(BC):
                # sync (HWDGE) DMA in f32, then GpSimd converts to bf16
                # (1-input GpSimd ops run at line rate; DVE stays free).
                xf = sb.tile([128, CH * D], f32, tag="xf", bufs=2, name=f"xf{b}")
                xb = sb.tile([128, CH * D], bf16, tag="xb", bufs=2, name=f"xb{b}")
                xf3 = xf.rearrange("p (c d) -> p c d", d=D)
                xs3 = x_in[b].rearrange("(c p) d -> p c d", p=128)
                half = CH // 2
                nc.sync.dma_start(out=xf3[:, :half, :], in_=xs3[:, :half, :])
                nc.sync.dma_start(out=xf3[:, half:, :], in_=xs3[:, half:, :])
                if b == 0:
                    # consts + z/zt loads queue AFTER the first x DMAs so
                    # they don't delay the critical path.
                    maskneg = sb.tile_from(maskneg_d[:, :], name="maskneg_sb")
                    eyeb = sb.tile_from(eyeb_d[:, :], name="eyeb_sb")
                    negshifti = sb.tile_from(negshifti_d[:, :],
                                             name="negshifti_sb")
                    sel4 = sb.tile_from(sel4_d[:, :], name="sel4_sb")
                    ones4 = sb.tile_from(ones4_d[:, :], name="ones4_sb")
                    ones1x32 = sb.tile_from(ones1x32_d[:, :],
                                            name="ones1x32_sb")
                    z_sb = sb.tile([B, D], f32, name="z_sb")
                    nc.sync.dma_start(out=z_sb[:, :], in_=z_in[:, :])
                    zts = []
                    for k in range(4):
                        zt_k = sb.tile([128, B], f32, name=f"zt{k}_sb")
                        nc.sync.dma_start(out=zt_k[:, :],
                                          in_=zt_in[k * 128:(k + 1) * 128, :])
                        zts.append(zt_k)
                q = CH * D // 4
                for qi in range(4):
                    nc.vector.tensor_copy(out=xb[:, qi * q:(qi + 1) * q],
                                          in_=xf[:, qi * q:(qi + 1) * q])
                for cch in range(CH):
                    xc = xb[:, cch * D:(cch + 1) * D]
                    for i in range(4):
                        nc.tensor.matmul(
                            g[i][:, :],
                            lhsT=xc[:, i * 128:(i + 1) * 128],
                            rhs=xc[:, 128 * i:],
                            start=(b == 0 and cch == 0), stop=False,
                        )
                    nc.tensor.matmul(
                        srow[:, :],
                        lhsT=sel4[:, 4 * b:4 * (b + 1)],
                        rhs=xc,
                        start=(b == 0 and cch == 0),
                        stop=(b == BC - 1 and cch == CH - 1),
                    )

            s_bf = sb.tile([BC, D], bf16, name="s_bf")
            nc.vector.tensor_copy(out=s_bf[:, :], in_=srow[:, :])
            sneg = sb.tile([BC, D], bf16, name="sneg")
            nc.vector.tensor_scalar_mul(sneg[:, :], srow[:, :], -1.0 / T)
            for i in range(4):
                nc.tensor.matmul(
                    g[i][:, :],
                    lhsT=sneg[:, i * 128:(i + 1) * 128],
                    rhs=s_bf[:, 128 * i:],
                    start=False, stop=True,
                )
            mrow = ps.tile([1, D], f32, tag="mrow", bufs=1, name="mrow")
            nc.tensor.matmul(mrow[:, :], lhsT=ones4[:, :], rhs=s_bf[:, :],
                             start=True, stop=True)

            # pack (PSUM - shift*I) to bf16
            arin_sb = sb.tile([128, AR_COLS], bf16, name="arin_sb")
            for i in range(4):
                cs = sum(W[:i])
                nc.vector.tensor_add(
                    out=arin_sb[:, cs:cs + 128],
                    in0=g[i][:, 0:128],
                    in1=negshifti[:, :],
                )
                if W[i] > 128:
                    nc.vector.tensor_copy(
                        out=arin_sb[:, cs + 128:cs + W[i]],
                        in_=g[i][:, 128:W[i]],
                    )
            arm_sb = sb.tile([1, AR_COLS], bf16, name="arm_sb")
            nc.vector.memset(arm_sb[:, D:], 0.0)
            nc.vector.tensor_copy(out=arm_sb[:, 0:D], in_=mrow[:, :])

        # ---- AllReduce ----
        ar_in = dr.tile([129, AR_COLS], bf16, name="ar_in")
        ar_out = dr.tile([129, AR_COLS], bf16, addr_space="Shared", name="ar_out")
        nc.sync.dma_start(out=ar_in[0:128, :], in_=arin_sb[:, :])
        nc.sync.dma_start(out=ar_in[128:129, :], in_=arm_sb[:, :])
        nc.gpsimd.collective_compute(
            "AllReduce",
            mybir.AluOpType.add,
            replica_groups=rg,
            ins=[ar_in[:, :].opt()],
            outs=[ar_out[:, :].opt()],
        )

        # ---- unpack: -E strips in bf16, masked fp32 E for the final round ----
        ebn, em = [], []
        for i in range(4):
            cs = sum(W[:i])
            er = sb.tile([128, W[i]], bf16, name=f"er{i}")
            nc.sync.dma_start(out=er[:, :], in_=ar_out[0:128, cs:cs + W[i]])
            eb = sb.tile([128, W[i]], bf16, name=f"ebn{i}")
            nc.vector.tensor_scalar_mul(eb[:, :], er[:, :], -1.0 / DENOM)
            # em_i = (E/DENOM's scale) * (-mask) = -E*mask  (one fused DVE op)
            ef = sb.tile([128, W[i]], f32, name=f"em{i}")
            nc.vector.scalar_tensor_tensor(
                out=ef[:, :], in0=er[:, :], scalar=1.0 / DENOM,
                in1=maskneg[:, :W[i]], op0=mult, op1=mult)
            ebn.append(eb)
            em.append(ef)
        armo = sb.tile([1, D], bf16, name="armo")
        nc.sync.dma_start(out=armo[:, :], in_=ar_out[128:129, 0:D])
        mean_sb = sb.tile([1, D], f32, name="mean_sb")
        nc.vector.tensor_scalar_mul(mean_sb[:, :], armo[:, :], 1.0 / (B * T))

        # ---- phase B: Cholesky fixed-point iteration + affine ----
        with tc.tile_pool(name="psB", space="PSUM", bufs=1) as ps:
            Y = None
            for rnd in range(N_BF16_ROUNDS + 1):
                last = rnd == N_BF16_ROUNDS
                newY = []
                for i in range(4):
                    p = ps.tile([128, W[i]], f32, tag="it", bufs=4,
                                name=f"it{rnd}_{i}")
                    first = True
                    if Y is not None:
                        for k in range(i + 1):
                            lo = 128 * (i - k)
                            nc.tensor.matmul(
                                p[:, :],
                                lhsT=Y[k][:, lo:lo + 128],
                                rhs=Y[k][:, lo:],
                                start=first, stop=(last and k == i),
                            )
                            first = False
                    if not last:
                        # fold -E into the accumulation via identity matmul
                        nc.tensor.matmul(p[:, :], lhsT=eyeb[:, :],
                                         rhs=ebn[i][:, :],
                                         start=first, stop=True)
                        ny = sb.tile([128, W[i]], bf16, tag="y", bufs=8,
                                     name=f"y{rnd}_{i}")
                        # psum = Y^T Y - E;  Y_new = -Phi(psum) = psum * (-mask)
                        nc.vector.tensor_tensor(out=ny[:, :], in0=p[:, :],
                                                in1=maskneg[:, :W[i]], op=mult)
                    else:
                        # fp32 refinement: Y = (E - Y^T Y) * mask
                        #   = psum*(-mask) - (-E*mask) = psum*(-mask) - em
                        ny = sb.tile([128, W[i]], f32, tag="yf", bufs=4,
                                     name=f"y{rnd}_{i}")
                        nc.vector.tensor_tensor(out=ny[:, :], in0=p[:, :],
                                                in1=maskneg[:, :W[i]], op=mult)
                        nc.vector.tensor_sub(ny[:, :], ny[:, :], em[i][:, :])
                    newY.append(ny)
                Y = newY

            # affine: out = z + z @ Y + mean  (fp32 matmuls; cheap)
            aff = ps.tile([B, D], f32, tag="aff", bufs=1, name="aff")
            for k in range(4):
                nc.tensor.matmul(
                    aff[:, 128 * k:],
                    lhsT=zts[k][:, :],
                    rhs=Y[k][:, :],
                    start=(k == 0), stop=False,
                )
            nc.tensor.matmul(aff[:, :], lhsT=ones1x32[:, :], rhs=mean_sb[:, :],
                             start=False, stop=True)
            out_sb = sb.tile([B, D], f32, name="out_sb")
            nc.vector.tensor_add(out=out_sb[:, :], in0=aff[:, :], in1=z_sb[:, :])
            nc.sync.dma_start(out=out_ext[:, :], in_=out_sb[:, :])

    nc.finalize()  # Bacc: runs event-sem splitting + register allocation
    return nc


_NC_CACHE = {}


def _get_nc():
    if "nc" not in _NC_CACHE:
        _NC_CACHE["nc"] = _build_nc()
    return _NC_CACHE["nc"]


def _in_maps(x, z):
    zt = np.ascontiguousarray(z.T)
    return [
        {"x": np.ascontiguousarray(x[c * BC:(c + 1) * BC]), "z": z, "zt": zt}
        for c in range(NCORES)
    ]


def kernel(x: np.ndarray, z: np.ndarray) -> np.ndarray:
    from concourse.bass_utils import run_bass_kernel_spmd

    x = np.ascontiguousarray(np.asarray(x, dtype=np.float32))
    z = np.ascontiguousarray(np.asarray(z, dtype=np.float32))
    nc = _get_nc()
    res = run_bass_kernel_spmd(nc, _in_maps(x, z), core_ids=list(range(NCORES)))
    return np.asarray(res.results[0]["out"], dtype=np.float32)


# revision 15
# speedup vs baseline: 1.4670x; 1.1014x over previous
"""Trainium2 Bass kernel for nn_BiasVectorsBlock (MVN sampling block).

Computes, for x [32, 2048, 512] and z [32, 512]:
    mean = mean(x, axis=(0,1))
    cov  = mean_b( xc_b^T xc_b / (T-1) ),  xc_b = x_b - mean_t(x_b)
    L    = cholesky(cov);  out = mean + z @ L^T

Strategy (8 NeuronCores, data-parallel over B):
  - core c DMA-loads its 4 batches once, casting f32 -> bf16 in the DMA.
    TensorE accumulates the Gram matrix G_c = sum x^T x (upper-triangle
    strips) and per-batch column sums S_c (selector-column matmuls) in
    PSUM across all 64 chunks.
  - correction -S_c^T S_c / T is accumulated into the same PSUM banks, so
    PSUM holds sum_b (x_b - mean_b)^T (x_b - mean_b) for the core's
    batches (bf16 inputs, fp32 accumulation).
  - subtract (T-1)*B/8 * I so the AllReduce payload is zero-centered
    (bf16-safe), pack to bf16, one AllReduce (~330 KB).
  - every core computes E = cov - I and runs the sqrt-free Cholesky
    fixed-point iteration  Y <- Phi_u(E - Y^T Y)  (Phi_u = strict upper +
    half diagonal; at the fixed point cov = (I+Y)^T (I+Y), i.e. R = I+Y
    is the upper Cholesky factor; converges in ~5 rounds since
    ||E||_2 ~ 0.18).  Rounds 0-3 in bf16 with E folded into PSUM via an
    identity matmul; the last round re-solves against fp32 E on VectorE.
  - out = z + (z @ Y) + mean via 4 fp32 z^T-chunk matmuls + a K=1
    ones-matmul that broadcasts mean into PSUM.
"""

import os
import sys

for _p in ("/opt/trn_rl_repo",):
    if _p not in sys.path and os.path.isdir(_p):
        sys.path.insert(0, _p)

import numpy as np

B, T, D = 32, 2048, 512
NCORES = 8
BC = B // NCORES          # batches per core
CH = T // 128             # 128-row chunks per batch
DENOM = (T - 1) * B       # cov denominator
SHIFT = DENOM / NCORES    # identity shift per core, so AR payload is zero-mean
W = [512, 384, 256, 128]  # upper-strip widths (strip i: rows 128i.., cols 128i..512)
N_BF16_ROUNDS = 4
AR_COLS = sum(W)          # 1280 packed columns


def _build_nc():
    import concourse.bacc as bacc
    import concourse.mybir as mybir
    import ml_dtypes
    from concourse.tile import TileContext

    f32 = mybir.dt.float32
    bf16 = mybir.dt.bfloat16
    mult = mybir.AluOpType.mult

    # Bacc (not raw Bass): its generate_event_semaphores pass splits
    # multi-wait instructions, which DMA opcodes require on TRN2.
    nc = bacc.Bacc(None, num_devices=NCORES)

    x_in = nc.declare_dram_parameter("x", [BC, T, D], f32, isOutput=False)
    z_in = nc.declare_dram_parameter("z", [B, D], f32, isOutput=False)
    zt_in = nc.declare_dram_parameter("zt", [D, B], f32, isOutput=False)
    out_ext = nc.declare_dram_parameter("out", [B, D], f32, isOutput=True)

    # ---- constants (embedded in the NEFF) ----
    # -Phi mask, shared by all strips: local cols 0:128 hold the diagonal
    # block (strict-upper -> -1, diag -> -0.5, lower -> 0); cols 128:512 -> -1.
    m = np.zeros((128, 512), np.float32)
    m[:, 128:] = -1.0
    r, c = np.indices((128, 128))
    m[:, :128] = np.where(c > r, -1.0, np.where(c == r, -0.5, 0.0)).astype(np.float32)
    maskneg_d = nc.inline_tensor(m, name="maskneg")

    eye = np.eye(128, dtype=np.float32)
    eyeb_d = nc.inline_tensor(eye.astype(ml_dtypes.bfloat16), name="eyeb")
    negshifti_d = nc.inline_tensor((-SHIFT) * eye, name="negshifti")
    sel = np.zeros((128, 4 * BC), np.float32)
    for b in range(BC):
        sel[:, 4 * b + b] = 1.0  # batch b's ones-column -> psum row b
    sel4_d = nc.inline_tensor(sel.astype(ml_dtypes.bfloat16), name="sel4")
    ones4_d = nc.inline_tensor(np.ones((BC, 1), ml_dtypes.bfloat16), name="ones4")
    ones1x32_d = nc.inline_tensor(np.ones((1, B), np.float32), name="ones1x32")

    rg = [list(range(NCORES))]

    with TileContext(nc) as tc, \
            tc.tile_pool(name="sb", bufs=1) as sb, \
            tc.tile_pool(name="dr", space="DRAM", bufs=1) as dr:

        # ---- phase A: Gram strips + per-batch column sums ----
        with tc.tile_pool(name="psA", space="PSUM", bufs=1) as ps:
            g = [ps.tile([128, W[i]], f32, tag=f"g{i}", bufs=1, name=f"g{i}")
                 for i in range(4)]
            srow = ps.tile([BC, D], f32, tag="srow", bufs=1, name="srow")
            for b in range(BC):
                # sync (HWDGE) DMA in f32, then GpSimd converts to bf16
                # (1-input GpSimd ops run at line rate; DVE stays free).
                xf = sb.tile([128, CH * D], f32, tag="xf", bufs=3, name=f"xf{b}")
                xb = sb.tile([128, CH * D], bf16, tag="xb", bufs=2, name=f"xb{b}")
                xf3 = xf.rearrange("p (c d) -> p c d", d=D)
                xs3 = x_in[b].rearrange("(c p) d -> p c d", p=128)
                half = CH // 2
                nc.sync.dma_start(out=xf3[:, :half, :], in_=xs3[:, :half, :])
                nc.sync.dma_start(out=xf3[:, half:, :], in_=xs3[:, half:, :])
                if b == 0:
                    # consts + z/zt loads queue AFTER the first x DMAs so
                    # they don't delay the critical path.
                    maskneg = sb.tile_from(maskneg_d[:, :], name="maskneg_sb")
                    eyeb = sb.tile_from(eyeb_d[:, :], name="eyeb_sb")
                    negshifti = sb.tile_from(negshifti_d[:, :],
                                             name="negshifti_sb")
                    sel4 = sb.tile_from(sel4_d[:, :], name="sel4_sb")
                    ones4 = sb.tile_from(ones4_d[:, :], name="ones4_sb")
                    ones1x32 = sb.tile_from(ones1x32_d[:, :],
                                            name="ones1x32_sb")
                    z_sb = sb.tile([B, D], f32, name="z_sb")
                    nc.sync.dma_start(out=z_sb[:, :], in_=z_in[:, :])
                    zts = []
                    for k in range(4):
                        zt_k = sb.tile([128, B], f32, name=f"zt{k}_sb")
                        nc.sync.dma_start(out=zt_k[:, :],
                                          in_=zt_in[k * 128:(k + 1) * 128, :])
                        zts.append(zt_k)
                q = CH * D // 4
                for qi in range(4):
                    nc.vector.tensor_copy(out=xb[:, qi * q:(qi + 1) * q],
                                          in_=xf[:, qi * q:(qi + 1) * q])
                for cch in range(CH):
                    xc = xb[:, cch * D:(cch + 1) * D]
                    for i in range(4):
                        nc.tensor.matmul(
                            g[i][:, :],
                            lhsT=xc[:, i * 128:(i + 1) * 128],
                            rhs=xc[:, 128 * i:],
                            start=(b == 0 and cch == 0), stop=False,
                        )
                # column sums: binary folds on VectorE (bf16, partial sums
                # stay well inside bf16 range), then one selector matmul
                # folds the partition dim into srow's row b.
                f1 = sb.tile([128, 8 * D], bf16, tag="f1", bufs=2, name=f"f1_{b}")
                nc.vector.tensor_add(out=f1[:, :], in0=xb[:, :8 * D],
                                     in1=xb[:, 8 * D:])
                f2 = sb.tile([128, 4 * D], bf16, tag="f2", bufs=2, name=f"f2_{b}")
                nc.vector.tensor_add(out=f2[:, :], in0=f1[:, :4 * D],
                                     in1=f1[:, 4 * D:])
                f3 = sb.tile([128, 2 * D], bf16, tag="f3", bufs=2, name=f"f3_{b}")
                nc.vector.tensor_add(out=f3[:, :], in0=f2[:, :2 * D],
                                     in1=f2[:, 2 * D:])
                accb = sb.tile([128, D], bf16, tag="accb", bufs=2,
                               name=f"accb{b}")
                nc.vector.tensor_add(out=accb[:, :], in0=f3[:, :D],
                                     in1=f3[:, D:])
                nc.tensor.matmul(
                    srow[:, :],
                    lhsT=sel4[:, 4 * b:4 * (b + 1)],
                    rhs=accb[:, :],
                    start=(b == 0), stop=(b == BC - 1),
                )

            s_bf = sb.tile([BC, D], bf16, name="s_bf")
            nc.vector.tensor_copy(out=s_bf[:, :], in_=srow[:, :])
            sneg = sb.tile([BC, D], bf16, name="sneg")
            nc.vector.tensor_scalar_mul(sneg[:, :], srow[:, :], -1.0 / T)
            for i in range(4):
                nc.tensor.matmul(
                    g[i][:, :],
                    lhsT=sneg[:, i * 128:(i + 1) * 128],
                    rhs=s_bf[:, 128 * i:],
                    start=False, stop=True,
                )
            mrow = ps.tile([1, D], f32, tag="mrow", bufs=1, name="mrow")
            nc.tensor.matmul(mrow[:, :], lhsT=ones4[:, :], rhs=s_bf[:, :],
                             start=True, stop=True)

            # pack (PSUM - shift*I) to bf16
            arin_sb = sb.tile([128, AR_COLS], bf16, name="arin_sb")
            for i in range(4):
                cs = sum(W[:i])
                nc.vector.tensor_add(
                    out=arin_sb[:, cs:cs + 128],
                    in0=g[i][:, 0:128],
                    in1=negshifti[:, :],
                )
                if W[i] > 128:
                    nc.vector.tensor_copy(
                        out=arin_sb[:, cs + 128:cs + W[i]],
                        in_=g[i][:, 128:W[i]],
                    )
            arm_sb = sb.tile([1, AR_COLS], bf16, name="arm_sb")
            nc.vector.memset(arm_sb[:, D:], 0.0)
            nc.vector.tensor_copy(out=arm_sb[:, 0:D], in_=mrow[:, :])

        # ---- AllReduce ----
        ar_in = dr.tile([129, AR_COLS], bf16, name="ar_in")
        ar_out = dr.tile([129, AR_COLS], bf16, addr_space="Shared", name="ar_out")
        nc.sync.dma_start(out=ar_in[0:128, :], in_=arin_sb[:, :])
        nc.sync.dma_start(out=ar_in[128:129, :], in_=arm_sb[:, :])
        nc.gpsimd.collective_compute(
            "AllReduce",
            mybir.AluOpType.add,
            replica_groups=rg,
            ins=[ar_in[:, :].opt()],
            outs=[ar_out[:, :].opt()],
        )

        # keep the PE's HAM clock warm through the AllReduce: a chain of
        # fp32 matmuls (4 cyc/row) gated on the AR input pack, accumulating
        # into a scratch PSUM bank nobody reads.
        with tc.tile_pool(name="psW", space="PSUM", bufs=1) as psw:
            warmsrc = sb.tile([128, D], f32, name="warmsrc")
            nc.vector.tensor_copy(out=warmsrc[:, :], in_=arin_sb[:, 0:D])
            warmps = psw.tile([128, D], f32, tag="warm", bufs=1, name="warmps")
            for wi in range(36):
                nc.tensor.matmul(warmps[:, :], lhsT=warmsrc[:, 0:128],
                                 rhs=warmsrc[:, :],
                                 start=(wi == 0), stop=(wi == 35))
            nc.vector.tensor_scalar_mul(warmsrc[:, 0:1], warmps[:, 0:1], 0.0)

        # ---- unpack: -E strips in bf16, masked fp32 E for the final round ----
        ebn, em = [], []
        for i in range(4):
            cs = sum(W[:i])
            er = sb.tile([128, W[i]], bf16, name=f"er{i}")
            nc.sync.dma_start(out=er[:, :], in_=ar_out[0:128, cs:cs + W[i]])
            eb = sb.tile([128, W[i]], bf16, name=f"ebn{i}")
            nc.vector.tensor_scalar_mul(eb[:, :], er[:, :], -1.0 / DENOM)
            # em_i = (E/DENOM's scale) * (-mask) = -E*mask  (one fused DVE op)
            ef = sb.tile([128, W[i]], f32, name=f"em{i}")
            nc.vector.scalar_tensor_tensor(
                out=ef[:, :], in0=er[:, :], scalar=1.0 / DENOM,
                in1=maskneg[:, :W[i]], op0=mult, op1=mult)
            ebn.append(eb)
            em.append(ef)
        armo = sb.tile([1, D], bf16, name="armo")
        nc.sync.dma_start(out=armo[:, :], in_=ar_out[128:129, 0:D])
        mean_sb = sb.tile([1, D], f32, name="mean_sb")
        nc.vector.tensor_scalar_mul(mean_sb[:, :], armo[:, :], 1.0 / (B * T))

        # ---- phase B: Cholesky fixed-point iteration + affine ----
        with tc.tile_pool(name="psB", space="PSUM", bufs=1) as ps:
            # round 0 is Y = Phi(E) = ebn * maskneg -- no matmul needed
            Y = []
            for i in range(4):
                y0 = sb.tile([128, W[i]], bf16, tag="y", bufs=8, name=f"y0_{i}")
                nc.vector.tensor_tensor(out=y0[:, :], in0=ebn[i][:, :],
                                        in1=maskneg[:, :W[i]], op=mult)
                Y.append(y0)
            for rnd in range(1, N_BF16_ROUNDS + 1):
                last = rnd == N_BF16_ROUNDS
                newY = []
                for i in range(4):
                    p = ps.tile([128, W[i]], f32, tag="it", bufs=4,
                                name=f"it{rnd}_{i}")
                    first = True
                    if True:
                        for k in range(i + 1):
                            lo = 128 * (i - k)
                            nc.tensor.matmul(
                                p[:, :],
                                lhsT=Y[k][:, lo:lo + 128],
                                rhs=Y[k][:, lo:],
                                start=first, stop=(last and k == i),
                            )
                            first = False
                    if not last:
                        # fold -E into the accumulation via identity matmul
                        nc.tensor.matmul(p[:, :], lhsT=eyeb[:, :],
                                         rhs=ebn[i][:, :],
                                         start=first, stop=True)
                        ny = sb.tile([128, W[i]], bf16, tag="y", bufs=8,
                                     name=f"y{rnd}_{i}")
                        # psum = Y^T Y - E;  Y_new = -Phi(psum) = psum * (-mask)
                        nc.vector.tensor_tensor(out=ny[:, :], in0=p[:, :],
                                                in1=maskneg[:, :W[i]], op=mult)
                    else:
                        # fp32 refinement: Y = (E - Y^T Y) * mask
                        #   = psum*(-mask) - (-E*mask) = psum*(-mask) - em
                        ny = sb.tile([128, W[i]], f32, tag="yf", bufs=4,
                                     name=f"y{rnd}_{i}")
                        nc.vector.tensor_tensor(out=ny[:, :], in0=p[:, :],
                                                in1=maskneg[:, :W[i]], op=mult)
                        nc.vector.tensor_sub(ny[:, :], ny[:, :], em[i][:, :])
                    newY.append(ny)
                Y = newY

            # affine: out = z + z @ Y + mean  (fp32 matmuls; cheap)
            aff = ps.tile([B, D], f32, tag="aff", bufs=1, name="aff")
            for k in range(4):
                nc.tensor.matmul(
                    aff[:, 128 * k:],
                    lhsT=zts[k][:, :],
                    rhs=Y[k][:, :],
                    start=(k == 0), stop=False,
                )
            nc.tensor.matmul(aff[:, :], lhsT=ones1x32[:, :], rhs=mean_sb[:, :],
                             start=False, stop=True)
            out_sb = sb.tile([B, D], f32, name="out_sb")
            nc.vector.tensor_add(out=out_sb[:, :], in0=aff[:, :], in1=z_sb[:, :])
            nc.sync.dma_start(out=out_ext[:, :], in_=out_sb[:, :])

    nc.finalize()  # Bacc: runs event-sem splitting + register allocation
    return nc


_NC_CACHE = {}


def _get_nc():
    if "nc" not in _NC_CACHE:
        _NC_CACHE["nc"] = _build_nc()
    return _NC_CACHE["nc"]


def _in_maps(x, z):
    zt = np.ascontiguousarray(z.T)
    return [
        {"x": np.ascontiguousarray(x[c * BC:(c + 1) * BC]), "z": z, "zt": zt}
        for c in range(NCORES)
    ]


def kernel(x: np.ndarray, z: np.ndarray) -> np.ndarray:
    from concourse.bass_utils import run_bass_kernel_spmd

    x = np.ascontiguousarray(np.asarray(x, dtype=np.float32))
    z = np.ascontiguousarray(np.asarray(z, dtype=np.float32))
    nc = _get_nc()
    res = run_bass_kernel_spmd(nc, _in_maps(x, z), core_ids=list(range(NCORES)))
    return np.asarray(res.results[0]["out"], dtype=np.float32)


# revision 16
# speedup vs baseline: 1.5122x; 1.0309x over previous
"""Trainium2 Bass kernel for nn_BiasVectorsBlock (MVN sampling block).

Computes, for x [32, 2048, 512] and z [32, 512]:
    mean = mean(x, axis=(0,1))
    cov  = mean_b( xc_b^T xc_b / (T-1) ),  xc_b = x_b - mean_t(x_b)
    L    = cholesky(cov);  out = mean + z @ L^T

Strategy (8 NeuronCores, data-parallel over B):
  - core c DMA-loads its 4 batches once, casting f32 -> bf16 in the DMA.
    TensorE accumulates the Gram matrix G_c = sum x^T x (upper-triangle
    strips) and per-batch column sums S_c (selector-column matmuls) in
    PSUM across all 64 chunks.
  - correction -S_c^T S_c / T is accumulated into the same PSUM banks, so
    PSUM holds sum_b (x_b - mean_b)^T (x_b - mean_b) for the core's
    batches (bf16 inputs, fp32 accumulation).
  - subtract (T-1)*B/8 * I so the AllReduce payload is zero-centered
    (bf16-safe), pack to bf16, one AllReduce (~330 KB).
  - every core computes E = cov - I and runs the sqrt-free Cholesky
    fixed-point iteration  Y <- Phi_u(E - Y^T Y)  (Phi_u = strict upper +
    half diagonal; at the fixed point cov = (I+Y)^T (I+Y), i.e. R = I+Y
    is the upper Cholesky factor; converges in ~5 rounds since
    ||E||_2 ~ 0.18).  Rounds 0-3 in bf16 with E folded into PSUM via an
    identity matmul; the last round re-solves against fp32 E on VectorE.
  - out = z + (z @ Y) + mean via 4 fp32 z^T-chunk matmuls + a K=1
    ones-matmul that broadcasts mean into PSUM.
"""

import os
import sys

for _p in ("/opt/trn_rl_repo",):
    if _p not in sys.path and os.path.isdir(_p):
        sys.path.insert(0, _p)

import numpy as np

B, T, D = 32, 2048, 512
NCORES = 8
BC = B // NCORES          # batches per core
CH = T // 128             # 128-row chunks per batch
DENOM = (T - 1) * B       # cov denominator
SHIFT = DENOM / NCORES    # identity shift per core, so AR payload is zero-mean
W = [512, 384, 256, 128]  # upper-strip widths (strip i: rows 128i.., cols 128i..512)
N_BF16_ROUNDS = 2
AR_COLS = sum(W)          # 1280 packed columns


def _build_nc():
    import concourse.bacc as bacc
    import concourse.mybir as mybir
    import ml_dtypes
    from concourse.tile import TileContext

    f32 = mybir.dt.float32
    bf16 = mybir.dt.bfloat16
    mult = mybir.AluOpType.mult

    # Bacc (not raw Bass): its generate_event_semaphores pass splits
    # multi-wait instructions, which DMA opcodes require on TRN2.
    nc = bacc.Bacc(None, num_devices=NCORES)

    x_in = nc.declare_dram_parameter("x", [BC, T, D], f32, isOutput=False)
    z_in = nc.declare_dram_parameter("z", [B, D], f32, isOutput=False)
    zt_in = nc.declare_dram_parameter("zt", [D, B], f32, isOutput=False)
    out_ext = nc.declare_dram_parameter("out", [B, D], f32, isOutput=True)

    # ---- constants (embedded in the NEFF) ----
    # -Phi mask, shared by all strips: local cols 0:128 hold the diagonal
    # block (strict-upper -> -1, diag -> -0.5, lower -> 0); cols 128:512 -> -1.
    m = np.zeros((128, 512), np.float32)
    m[:, 128:] = -1.0
    r, c = np.indices((128, 128))
    m[:, :128] = np.where(c > r, -1.0, np.where(c == r, -0.5, 0.0)).astype(np.float32)
    maskneg_d = nc.inline_tensor(m, name="maskneg")
    maskpd_d = nc.inline_tensor(-m / DENOM, name="maskpd")

    eye = np.eye(128, dtype=np.float32)
    eyeb_d = nc.inline_tensor(eye.astype(ml_dtypes.bfloat16), name="eyeb")
    negshifti_d = nc.inline_tensor((-SHIFT) * eye, name="negshifti")
    sel = np.zeros((128, 4 * BC), np.float32)
    for b in range(BC):
        sel[:, 4 * b + b] = 1.0  # batch b's ones-column -> psum row b
    sel4_d = nc.inline_tensor(sel.astype(ml_dtypes.bfloat16), name="sel4")
    ones4_d = nc.inline_tensor(np.ones((BC, 1), ml_dtypes.bfloat16), name="ones4")
    ones1x32_d = nc.inline_tensor(np.ones((1, B), np.float32), name="ones1x32")

    rg = [list(range(NCORES))]

    with TileContext(nc) as tc, \
            tc.tile_pool(name="sb", bufs=1) as sb, \
            tc.tile_pool(name="dr", space="DRAM", bufs=1) as dr:

        # ---- phase A: Gram strips + per-batch column sums ----
        with tc.tile_pool(name="psA", space="PSUM", bufs=1) as ps:
            g = [ps.tile([128, W[i]], f32, tag=f"g{i}", bufs=1, name=f"g{i}")
                 for i in range(4)]
            srow = ps.tile([BC, D], f32, tag="srow", bufs=1, name="srow")
            for b in range(BC):
                # sync (HWDGE) DMA in f32, then GpSimd converts to bf16
                # (1-input GpSimd ops run at line rate; DVE stays free).
                xf = sb.tile([128, CH * D], f32, tag="xf", bufs=3, name=f"xf{b}")
                xb = sb.tile([128, CH * D], bf16, tag="xb", bufs=2, name=f"xb{b}")
                xf3 = xf.rearrange("p (c d) -> p c d", d=D)
                xs3 = x_in[b].rearrange("(c p) d -> p c d", p=128)
                half = CH // 2
                nc.sync.dma_start(out=xf3[:, :half, :], in_=xs3[:, :half, :])
                nc.sync.dma_start(out=xf3[:, half:, :], in_=xs3[:, half:, :])
                if b == 0:
                    # consts + z/zt loads queue AFTER the first x DMAs so
                    # they don't delay the critical path.
                    maskneg = sb.tile_from(maskneg_d[:, :], name="maskneg_sb", forced_dma_engine=mybir.EngineType.Activation)
                    maskpd = sb.tile_from(maskpd_d[:, :], name="maskpd_sb", forced_dma_engine=mybir.EngineType.Activation)
                    eyeb = sb.tile_from(eyeb_d[:, :], name="eyeb_sb", forced_dma_engine=mybir.EngineType.Activation)
                    negshifti = sb.tile_from(negshifti_d[:, :], name="negshifti_sb", forced_dma_engine=mybir.EngineType.Activation)
                    sel4 = sb.tile_from(sel4_d[:, :], name="sel4_sb", forced_dma_engine=mybir.EngineType.Activation)
                    ones4 = sb.tile_from(ones4_d[:, :], name="ones4_sb", forced_dma_engine=mybir.EngineType.Activation)
                    ones1x32 = sb.tile_from(ones1x32_d[:, :], name="ones1x32_sb", forced_dma_engine=mybir.EngineType.Activation)
                    z_sb = sb.tile([B, D], f32, name="z_sb")
                    nc.scalar.dma_start(out=z_sb[:, :], in_=z_in[:, :])
                    zts = []
                    for k in range(4):
                        zt_k = sb.tile([128, B], f32, name=f"zt{k}_sb")
                        nc.scalar.dma_start(out=zt_k[:, :],
                                          in_=zt_in[k * 128:(k + 1) * 128, :])
                        zts.append(zt_k)
                q = CH * D // 4
                for qi in range(4):
                    nc.vector.tensor_copy(out=xb[:, qi * q:(qi + 1) * q],
                                          in_=xf[:, qi * q:(qi + 1) * q])
                for cch in range(CH):
                    xc = xb[:, cch * D:(cch + 1) * D]
                    for i in range(4):
                        nc.tensor.matmul(
                            g[i][:, :],
                            lhsT=xc[:, i * 128:(i + 1) * 128],
                            rhs=xc[:, 128 * i:],
                            start=(b == 0 and cch == 0), stop=False,
                        )
                # column sums: binary folds on VectorE (bf16, partial sums
                # stay well inside bf16 range), then one selector matmul
                # folds the partition dim into srow's row b.
                f1 = sb.tile([128, 8 * D], bf16, tag="f1", bufs=2, name=f"f1_{b}")
                nc.vector.tensor_add(out=f1[:, :], in0=xb[:, :8 * D],
                                     in1=xb[:, 8 * D:])
                f2 = sb.tile([128, 4 * D], bf16, tag="f2", bufs=2, name=f"f2_{b}")
                nc.vector.tensor_add(out=f2[:, :], in0=f1[:, :4 * D],
                                     in1=f1[:, 4 * D:])
                f3 = sb.tile([128, 2 * D], bf16, tag="f3", bufs=2, name=f"f3_{b}")
                nc.vector.tensor_add(out=f3[:, :], in0=f2[:, :2 * D],
                                     in1=f2[:, 2 * D:])
                accb = sb.tile([128, D], bf16, tag="accb", bufs=2,
                               name=f"accb{b}")
                nc.vector.tensor_add(out=accb[:, :], in0=f3[:, :D],
                                     in1=f3[:, D:])
                nc.tensor.matmul(
                    srow[:, :],
                    lhsT=sel4[:, 4 * b:4 * (b + 1)],
                    rhs=accb[:, :],
                    start=(b == 0), stop=(b == BC - 1),
                )

            s_bf = sb.tile([BC, D], bf16, name="s_bf")
            nc.vector.tensor_copy(out=s_bf[:, :], in_=srow[:, :])
            sneg = sb.tile([BC, D], bf16, name="sneg")
            nc.vector.tensor_scalar_mul(sneg[:, :], srow[:, :], -1.0 / T)
            for i in range(4):
                nc.tensor.matmul(
                    g[i][:, :],
                    lhsT=sneg[:, i * 128:(i + 1) * 128],
                    rhs=s_bf[:, 128 * i:],
                    start=False, stop=True,
                )
            mrow = ps.tile([1, D], f32, tag="mrow", bufs=1, name="mrow")
            nc.tensor.matmul(mrow[:, :], lhsT=ones4[:, :], rhs=s_bf[:, :],
                             start=True, stop=True)

            # pack (PSUM - shift*I) to bf16
            arin_sb = sb.tile([128, AR_COLS], bf16, name="arin_sb")
            for i in range(4):
                cs = sum(W[:i])
                nc.vector.tensor_add(
                    out=arin_sb[:, cs:cs + 128],
                    in0=g[i][:, 0:128],
                    in1=negshifti[:, :],
                )
                if W[i] > 128:
                    nc.vector.tensor_copy(
                        out=arin_sb[:, cs + 128:cs + W[i]],
                        in_=g[i][:, 128:W[i]],
                    )
            arm_sb = sb.tile([1, AR_COLS], bf16, name="arm_sb")
            nc.vector.memset(arm_sb[:, D:], 0.0)
            nc.vector.tensor_copy(out=arm_sb[:, 0:D], in_=mrow[:, :])

        # ---- AllReduce ----
        ar_in = dr.tile([129, AR_COLS], bf16, name="ar_in")
        ar_out = dr.tile([129, AR_COLS], bf16, addr_space="Shared", name="ar_out")
        nc.scalar.dma_start(out=ar_in[0:128, :], in_=arin_sb[:, :])
        nc.scalar.dma_start(out=ar_in[128:129, :], in_=arm_sb[:, :])
        nc.gpsimd.collective_compute(
            "AllReduce",
            mybir.AluOpType.add,
            replica_groups=rg,
            ins=[ar_in[:, :].opt()],
            outs=[ar_out[:, :].opt()],
        )

        # keep the PE's HAM clock warm through the AllReduce: a chain of
        # fp32 matmuls (4 cyc/row) gated on the AR input pack, accumulating
        # into a scratch PSUM bank nobody reads.
        with tc.tile_pool(name="psW", space="PSUM", bufs=1) as psw:
            warmsrc = sb.tile([128, D], f32, name="warmsrc")
            nc.vector.tensor_copy(out=warmsrc[:, :], in_=arin_sb[:, 0:D])
            warmps = psw.tile([128, D], f32, tag="warm", bufs=1, name="warmps")
            for wi in range(32):
                nc.tensor.matmul(warmps[:, :], lhsT=warmsrc[:, 0:128],
                                 rhs=warmsrc[:, :],
                                 start=(wi == 0), stop=(wi == 31))
            nc.vector.tensor_scalar_mul(warmsrc[:, 0:1], warmps[:, 0:1], 0.0)

        # ---- unpack: -E strips in bf16, masked fp32 E for the final round ----
        ebn, em, ebn_raw = [], [], []
        for i in range(4):
            cs = sum(W[:i])
            er = sb.tile([128, W[i]], bf16, name=f"er{i}")
            nc.scalar.dma_start(out=er[:, :], in_=ar_out[0:128, cs:cs + W[i]])
            eb = sb.tile([128, W[i]], bf16, name=f"ebn{i}")
            nc.vector.tensor_scalar_mul(eb[:, :], er[:, :], -1.0 / DENOM)
            # em_i = (E/DENOM's scale) * (-mask) = -E*mask  (one fused DVE op)
            ef = sb.tile([128, W[i]], f32, name=f"em{i}")
            nc.vector.scalar_tensor_tensor(
                out=ef[:, :], in0=er[:, :], scalar=1.0 / DENOM,
                in1=maskneg[:, :W[i]], op0=mult, op1=mult)
            ebn.append(eb)
            em.append(ef)
            ebn_raw.append(er)
        armo = sb.tile([1, D], bf16, name="armo")
        nc.scalar.dma_start(out=armo[:, :], in_=ar_out[128:129, 0:D])
        mean_sb = sb.tile([1, D], f32, name="mean_sb")
        nc.vector.tensor_scalar_mul(mean_sb[:, :], armo[:, :], 1.0 / (B * T))

        # ---- phase B: Cholesky fixed-point iteration + affine ----
        with tc.tile_pool(name="psB", space="PSUM", bufs=1) as ps:
            # round 0 is Y = Phi(E) = er * (mask/DENOM) -- no matmul needed
            Y = []
            for i in range(4):
                y0 = sb.tile([128, W[i]], bf16, tag="y", bufs=8, name=f"y0_{i}")
                nc.vector.tensor_tensor(out=y0[:, :], in0=ebn_raw[i][:, :],
                                        in1=maskpd[:, :W[i]], op=mult)
                Y.append(y0)
            for rnd in range(1, N_BF16_ROUNDS + 1):
                last = rnd == N_BF16_ROUNDS
                newY = []
                for i in range(4):
                    p = ps.tile([128, W[i]], f32, tag="it", bufs=4,
                                name=f"it{rnd}_{i}")
                    first = True
                    if True:
                        for k in range(i + 1):
                            lo = 128 * (i - k)
                            nc.tensor.matmul(
                                p[:, :],
                                lhsT=Y[k][:, lo:lo + 128],
                                rhs=Y[k][:, lo:],
                                start=first, stop=(last and k == i),
                            )
                            first = False
                    if not last:
                        # fold -E into the accumulation via identity matmul
                        nc.tensor.matmul(p[:, :], lhsT=eyeb[:, :],
                                         rhs=ebn[i][:, :],
                                         start=first, stop=True)
                        ny = sb.tile([128, W[i]], bf16, tag="y", bufs=8,
                                     name=f"y{rnd}_{i}")
                        # psum = Y^T Y - E;  Y_new = -Phi(psum) = psum * (-mask)
                        nc.vector.tensor_tensor(out=ny[:, :], in0=p[:, :],
                                                in1=maskneg[:, :W[i]], op=mult)
                    else:
                        # fp32 refinement: Y = (E - Y^T Y) * mask
                        #   = psum*(-mask) - (-E*mask) = psum*(-mask) - em
                        ny = sb.tile([128, W[i]], f32, tag="yf", bufs=4,
                                     name=f"y{rnd}_{i}")
                        nc.vector.tensor_tensor(out=ny[:, :], in0=p[:, :],
                                                in1=maskneg[:, :W[i]], op=mult)
                        nc.vector.tensor_sub(ny[:, :], ny[:, :], em[i][:, :])
                    newY.append(ny)
                Y = newY

            # affine: out = z + z @ Y + mean  (fp32 matmuls; cheap)
            aff = ps.tile([B, D], f32, tag="aff", bufs=1, name="aff")
            for k in range(4):
                nc.tensor.matmul(
                    aff[:, 128 * k:],
                    lhsT=zts[k][:, :],
                    rhs=Y[k][:, :],
                    start=(k == 0), stop=False,
                )
            nc.tensor.matmul(aff[:, :], lhsT=ones1x32[:, :], rhs=mean_sb[:, :],
                             start=False, stop=True)
            out_sb = sb.tile([B, D], f32, name="out_sb")
            nc.vector.tensor_add(out=out_sb[:, :], in0=aff[:, :], in1=z_sb[:, :])
            nc.scalar.dma_start(out=out_ext[:, :], in_=out_sb[:, :])

    nc.finalize()  # Bacc: runs event-sem splitting + register allocation
    return nc


_NC_CACHE = {}


def _get_nc():
    if "nc" not in _NC_CACHE:
        _NC_CACHE["nc"] = _build_nc()
    return _NC_CACHE["nc"]


def _in_maps(x, z):
    zt = np.ascontiguousarray(z.T)
    return [
        {"x": np.ascontiguousarray(x[c * BC:(c + 1) * BC]), "z": z, "zt": zt}
        for c in range(NCORES)
    ]


def kernel(x: np.ndarray, z: np.ndarray) -> np.ndarray:
    from concourse.bass_utils import run_bass_kernel_spmd

    x = np.ascontiguousarray(np.asarray(x, dtype=np.float32))
    z = np.ascontiguousarray(np.asarray(z, dtype=np.float32))
    nc = _get_nc()
    res = run_bass_kernel_spmd(nc, _in_maps(x, z), core_ids=list(range(NCORES)))
    return np.asarray(res.results[0]["out"], dtype=np.float32)
